# revision 22
# baseline (speedup 1.0000x reference)
import os
import numpy as np

# nn_PixelflyLinear: y = (x @ w1.T) @ w2.T + b + butterfly_matmul(x, weight, flat_idx)
# Data-parallel over tokens: 8 cores x 512 tokens, weights replicated.
# Device computes yT (out_f on partitions, tokens on free dim); host transposes.

TOKENS, IN_F, OUT_F, LOWRANK = 4096, 4096, 4096, 256
BLOCK, ACTIVE, NB = 256, 5, 16
NCORES = 8
TPC = TOKENS // NCORES          # 512 tokens per core
NG = OUT_F // 128               # 32 output half-block groups
NXT = IN_F // 128               # 32 input tiles
NSLOT = 12                      # 10 butterfly + 2 lowrank lhsT slots per group

_CACHE = {}
LAST = {"exec_time_ns": None}


def _derive_xtile_idx(flat):
    xtile_idx = np.zeros((NG, 10), np.int64)
    for ob in range(NB):
        for j in range(ACTIVE):
            m = int(flat[ob, j])
            q = m // ACTIVE
            for rh in range(2):
                for kh in range(2):
                    xtile_idx[ob * 2 + rh, j * 2 + kh] = q * 2 + kh
    return xtile_idx


def _build(xtile_idx):
    import concourse.bacc as bacc
    import concourse.mybir as mybir
    import concourse.tile as tile

    nc = bacc.Bacc("TRN2", target_bir_lowering=False, debug=False,
                   num_devices=NCORES)
    dt = mybir.dt

    LEADS = 6
    YCH = 4                                  # groups per y-out DMA
    # x-tile chunks (tapered: small first for early PE start, fat later)
    XCH = [(0, 1), (1, 4), (4, 10), (10, 18), (18, 26), (26, 32)]
    # w1 slot ranges per DMA piece (slot = i*2+lh, 64 slots total)
    W1CH = [(0, 8), (8, 32), (32, 64)]

    xpack_d = nc.dram_tensor("xpack", [128, NXT * TPC], dt.float16,
                             kind="ExternalInput")
    w1_d = nc.dram_tensor("w1pack", [128, 64 * 128], dt.float16,
                          kind="ExternalInput")
    g_d = nc.dram_tensor("gpack", [NG // 2, 128, 2 * NSLOT * 128], dt.float16,
                         kind="ExternalInput")
    b_d = nc.dram_tensor("bpack", [128, NG], dt.float32, kind="ExternalInput")
    y_d = nc.dram_tensor("y", [NG // YCH, 128, YCH * TPC], dt.float16,
                         kind="ExternalOutput")

    with tile.TileContext(nc) as tc:
        with (
            tc.tile_pool(name="res", bufs=1) as res_pool,
            tc.tile_pool(name="gstream", bufs=3) as gpool,
            tc.tile_pool(name="ypool", bufs=3) as ypool,
            tc.tile_pool(name="upsum", bufs=1, space="PSUM") as upsum,
            tc.tile_pool(name="gpsum", bufs=6, space="PSUM") as gpsum,
        ):
            bt = res_pool.tile([128, NG], dt.float32, tag="b")
            nc.sync.dma_start(bt[:], b_d[:])

            xch = [None] * len(XCH)          # SBUF chunk tiles
            w1p = [None] * len(W1CH)
            gpt = [None] * (NG // 2)         # gpack pair tiles
            accs = [None] * NG

            def dma_x(j):
                lo, hi = XCH[j]
                t = res_pool.tile([128, (hi - lo) * TPC], dt.float16,
                                  tag=f"xc{j}", name=f"xc{j}")
                nc.sync.dma_start(t[:], xpack_d[:, lo * TPC:hi * TPC])
                xch[j] = t

            def dma_w1(k):
                lo, hi = W1CH[k]
                t = res_pool.tile([128, (hi - lo) * 128], dt.float16,
                                  tag=f"w1_{k}", name=f"w1p{k}")
                nc.sync.dma_start(t[:], w1_d[:, lo * 128:hi * 128])
                w1p[k] = t

            def dma_gp(p):
                gt = gpool.tile([128, 2 * NSLOT * 128], dt.float16, tag="g",
                                name=f"gp{p}")
                nc.sync.dma_start(gt[:], g_d[p])
                gpt[p] = gt

            def xslice(i):
                for j, (lo, hi) in enumerate(XCH):
                    if lo <= i < hi:
                        return xch[j][:, (i - lo) * TPC:(i - lo + 1) * TPC]

            def w1slice(slot):
                for k, (lo, hi) in enumerate(W1CH):
                    if lo <= slot < hi:
                        return w1p[k][:, (slot - lo) * 128:(slot - lo + 1) * 128]

            def gslice(g, s):
                off = (g % 2) * NSLOT * 128
                return gpt[g // 2][:, off + s * 128:off + (s + 1) * 128]

            # DMA issue order for the stream-in phase (few fat DMAs);
            # pos index doubles as the availability ordinal below
            order = ["w1:0", "x:0", "x:1", "g:0", "x:2", "w1:1", "g:1",
                     "x:3", "g:2", "w1:2", "x:4", "x:5"]
            pos = {}
            for p, item in enumerate(order):
                kind, idx = item.split(":")
                {"x": dma_x, "w1": dma_w1, "g": dma_gp}[kind](int(idx))
                pos[item] = p

            def xpos(i):
                for j, (lo, hi) in enumerate(XCH):
                    if lo <= i < hi:
                        return pos[f"x:{j}"]

            def w1pos(slot):
                for k, (lo, hi) in enumerate(W1CH):
                    if lo <= slot < hi:
                        return pos[f"w1:{k}"]

            u_ps = [upsum.tile([128, TPC], dt.float32, tag=f"u{lh}",
                               name=f"ups{lh}") for lh in range(2)]

            # merged emission: u matmuls + lead-group butterfly matmuls,
            # sorted by the DMA position that unblocks them
            events = []
            for i in range(NXT):
                av = max(xpos(i), w1pos(i * 2 + 1))
                events.append((av, 0, ("u", i)))
            for g in range(LEADS):
                gav = pos[f"g:{g // 2}"]
                slots = sorted(
                    range(10),
                    key=lambda s: (max(xpos(int(xtile_idx[g, s])), gav), s))
                first = True
                for s in slots:
                    av = max(xpos(int(xtile_idx[g, s])), gav)
                    events.append((av, 1, ("bf", g, s, first)))
                    first = False
            events.sort(key=lambda e: (e[0], e[1]))

            for av, pri, ev in events:
                if ev[0] == "u":
                    i = ev[1]
                    for lh in range(2):
                        nc.tensor.matmul(u_ps[lh][:], w1slice(i * 2 + lh),
                                         xslice(i),
                                         start=(i == 0), stop=(i == NXT - 1))
                else:
                    _, g, s, first = ev
                    if accs[g] is None:
                        accs[g] = gpsum.tile([128, TPC], dt.float32,
                                             tag="acc", name=f"acc{g}")
                    nc.tensor.matmul(accs[g][:], gslice(g, s),
                                     xslice(int(xtile_idx[g, s])),
                                     start=first, stop=False)

            u_sb = []
            for lh in range(2):
                ut = res_pool.tile([128, TPC], dt.float16, tag=f"usb{lh}",
                                   name=f"usb{lh}")
                nc.vector.tensor_copy(ut[:], u_ps[lh][:])
                u_sb.append(ut)

            ycur = [None]

            def close_group(g):
                for lh in range(2):
                    nc.tensor.matmul(accs[g][:], gslice(g, 10 + lh),
                                     u_sb[lh][:],
                                     start=False, stop=(lh == 1))
                if g % YCH == 0:
                    ycur[0] = ypool.tile([128, YCH * TPC], dt.float16,
                                         tag="y", name=f"yc{g // YCH}")
                c = g % YCH
                nc.vector.tensor_scalar_add(
                    ycur[0][:, c * TPC:(c + 1) * TPC], accs[g][:],
                    bt[:, g:g + 1])
                if c == YCH - 1:
                    nc.sync.dma_start(y_d[g // YCH], ycur[0][:])

            for g in range(LEADS):
                close_group(g)

            for g in range(LEADS, NG):
                if g % 2 == 0:
                    dma_gp(g // 2)
                accs[g] = gpsum.tile([128, TPC], dt.float32, tag="acc",
                                     name=f"acc{g}")
                for s in range(10):
                    nc.tensor.matmul(accs[g][:], gslice(g, s),
                                     xslice(int(xtile_idx[g, s])),
                                     start=(s == 0), stop=False)
                close_group(g)

    nc.compile()
    return nc


def _pack_weights(weight, w1, w2, b, flat):
    r2 = np.arange(BLOCK)
    gpack = np.empty((NG, 128, NSLOT * 128), np.float16)
    # packed below into pairs [NG//2, 128, 2*NSLOT*128] for 6KB DMA rows
    for ob in range(NB):
        for j in range(ACTIVE):
            m = int(flat[ob, j])
            q, a2 = m // ACTIVE, m % ACTIVE
            k = a2 * BLOCK + r2
            Wblk = weight[q * BLOCK + k // ACTIVE, k % ACTIVE, :]  # [r2, c]
            for rh in range(2):
                g = ob * 2 + rh
                for kh in range(2):
                    s = j * 2 + kh
                    gpack[g, :, s * 128:(s + 1) * 128] = \
                        Wblk[rh * 128:(rh + 1) * 128,
                             kh * 128:(kh + 1) * 128].T
    for g in range(NG):
        for lh in range(2):
            s = 10 + lh
            gpack[g, :, s * 128:(s + 1) * 128] = \
                w2[g * 128:(g + 1) * 128, lh * 128:(lh + 1) * 128].T
    gpairs = np.ascontiguousarray(
        gpack.reshape(NG // 2, 2, 128, NSLOT * 128)
             .transpose(0, 2, 1, 3)
             .reshape(NG // 2, 128, 2 * NSLOT * 128))
    w1sb = np.ascontiguousarray(
        w1.reshape(2, 128, 32, 128).transpose(2, 0, 3, 1)
          .reshape(64, 128, 128).transpose(1, 0, 2)
          .reshape(128, 64 * 128)).astype(np.float16)
    bpack = np.ascontiguousarray(b.reshape(NG, 128).T)
    return gpairs, w1sb, bpack


def _ensure_axon_hooks():
    # Some images lack antenv.axon_hooks; bass_utils imports it on the
    # trace path. Provide a stub so trace degrades gracefully.
    import sys
    import types
    try:
        import antenv.axon_hooks  # noqa: F401
        return
    except ImportError:
        pass
    mod = types.ModuleType("antenv.axon_hooks")
    mod._hook = None
    mod.set_axon_ntff_profile_hook = lambda h: setattr(mod, "_hook", h)
    mod.get_axon_ntff_profile_hook = lambda: mod._hook
    sys.modules["antenv.axon_hooks"] = mod
    try:
        import antenv
        antenv.axon_hooks = mod
    except ImportError:
        pass


def kernel(x, weight, w1, w2, b, butterfly_flat_indices):
    _ensure_axon_hooks()
    from concourse.bass_utils import run_bass_kernel_spmd

    x = np.ascontiguousarray(x, np.float32)
    weight = np.ascontiguousarray(weight, np.float32)
    w1 = np.ascontiguousarray(w1, np.float32)
    w2 = np.ascontiguousarray(w2, np.float32)
    b = np.ascontiguousarray(b, np.float32)
    flat = np.asarray(butterfly_flat_indices)

    xtile_idx = _derive_xtile_idx(flat)
    key = xtile_idx.tobytes()
    if key not in _CACHE:
        _CACHE[key] = _build(xtile_idx)
    nc = _CACHE[key]

    gpairs, w1sb, bpack = _pack_weights(weight, w1, w2, b, flat)
    in_maps = []
    for c in range(NCORES):
        xs = x[c * TPC:(c + 1) * TPC]
        xpack = np.ascontiguousarray(
            xs.T.reshape(NXT, 128, TPC).transpose(1, 0, 2)
              .reshape(128, NXT * TPC)).astype(np.float16)
        in_maps.append({"xpack": xpack, "w1pack": w1sb, "gpack": gpairs,
                        "bpack": bpack})

    trace = bool(int(os.environ.get("PIXELFLY_TRACE", "0")))
    res = run_bass_kernel_spmd(nc, in_maps, list(range(NCORES)), trace=trace)
    LAST["exec_time_ns"] = res.exec_time_ns
    LAST["results"] = res

    out = np.empty((TOKENS, OUT_F), np.float32)
    for c in range(NCORES):
        yc = res.results[c]["y"]  # [NG//4, 128, 4*TPC] fp16
        yfull = (yc.reshape(NG // 4, 128, 4, TPC).transpose(0, 2, 1, 3)
                   .reshape(OUT_F, TPC))
        out[c * TPC:(c + 1) * TPC] = yfull.T.astype(np.float32)
    return out


# revision 27
# speedup vs baseline: 1.0394x; 1.0394x over previous
import os
import numpy as np

# nn_PixelflyLinear: y = (x @ w1.T) @ w2.T + b + butterfly_matmul(x, weight, flat_idx)
# Data-parallel over tokens: 8 cores x 512 tokens, weights replicated.
# Device computes yT (out_f on partitions, tokens on free dim); host transposes.

TOKENS, IN_F, OUT_F, LOWRANK = 4096, 4096, 4096, 256
BLOCK, ACTIVE, NB = 256, 5, 16
NCORES = 8
TPC = TOKENS // NCORES          # 512 tokens per core
NG = OUT_F // 128               # 32 output half-block groups
NXT = IN_F // 128               # 32 input tiles
NSLOT = 12                      # 10 butterfly + 2 lowrank lhsT slots per group

_CACHE = {}
LAST = {"exec_time_ns": None}


def _derive_xtile_idx(flat):
    xtile_idx = np.zeros((NG, 10), np.int64)
    for ob in range(NB):
        for j in range(ACTIVE):
            m = int(flat[ob, j])
            q = m // ACTIVE
            for rh in range(2):
                for kh in range(2):
                    xtile_idx[ob * 2 + rh, j * 2 + kh] = q * 2 + kh
    return xtile_idx


def _build(xtile_idx):
    import concourse.bacc as bacc
    import concourse.mybir as mybir
    import concourse.tile as tile

    nc = bacc.Bacc("TRN2", target_bir_lowering=False, debug=False,
                   num_devices=NCORES)
    dt = mybir.dt

    LEADS = 6
    # x-tile chunks (tapered: small first for early PE start, fat later)
    XCH = [(0, 1), (1, 4), (4, 10), (10, 18), (18, 26), (26, 32)]
    # w1 slot ranges per DMA piece (slot = i*2+lh, 64 slots total)
    W1CH = [(0, 4), (4, 32), (32, 64)]
    # y-out group chunks (tapered at the end to shrink the drain tail)
    YCH = [(0, 4), (4, 8), (8, 12), (12, 16), (16, 20), (20, 24), (24, 28),
           (28, 30), (30, 31), (31, 32)]

    xpack_d = nc.dram_tensor("xpack", [128, NXT * TPC], dt.float16,
                             kind="ExternalInput")
    w1_d = nc.dram_tensor("w1pack", [128, 64 * 128], dt.float16,
                          kind="ExternalInput")
    g_d = nc.dram_tensor("gpack", [NG // 2, 128, 2 * NSLOT * 128], dt.float16,
                         kind="ExternalInput")
    b_d = nc.dram_tensor("bpack", [128, NG], dt.float32, kind="ExternalInput")
    y_d = nc.dram_tensor("y", [128, NG * TPC], dt.float16,
                         kind="ExternalOutput")

    with tile.TileContext(nc) as tc:
        with (
            tc.tile_pool(name="res", bufs=1) as res_pool,
            tc.tile_pool(name="upsum", bufs=1, space="PSUM") as upsum,
            tc.tile_pool(name="gpsum", bufs=6, space="PSUM") as gpsum,
        ):
            bt = res_pool.tile([128, NG], dt.float32, tag="b")
            nc.sync.dma_start(bt[:], b_d[:])

            xch = [None] * len(XCH)          # SBUF chunk tiles
            w1p = [None] * len(W1CH)
            gpt = [None] * (NG // 2)         # gpack pair tiles
            accs = [None] * NG

            def dma_x(j):
                lo, hi = XCH[j]
                t = res_pool.tile([128, (hi - lo) * TPC], dt.float16,
                                  tag=f"xc{j}", name=f"xc{j}")
                nc.sync.dma_start(t[:], xpack_d[:, lo * TPC:hi * TPC])
                xch[j] = t

            def dma_w1(k):
                lo, hi = W1CH[k]
                t = res_pool.tile([128, (hi - lo) * 128], dt.float16,
                                  tag=f"w1_{k}", name=f"w1p{k}")
                nc.sync.dma_start(t[:], w1_d[:, lo * 128:hi * 128])
                w1p[k] = t

            def dma_gp(p):
                gt = res_pool.tile([128, 2 * NSLOT * 128], dt.float16,
                                   tag=f"gp{p}", name=f"gp{p}")
                nc.sync.dma_start(gt[:], g_d[p])
                gpt[p] = gt

            def xslice(i):
                for j, (lo, hi) in enumerate(XCH):
                    if lo <= i < hi:
                        return xch[j][:, (i - lo) * TPC:(i - lo + 1) * TPC]

            def w1slice(slot):
                for k, (lo, hi) in enumerate(W1CH):
                    if lo <= slot < hi:
                        return w1p[k][:, (slot - lo) * 128:(slot - lo + 1) * 128]

            def gslice(g, s):
                off = (g % 2) * NSLOT * 128
                return gpt[g // 2][:, off + s * 128:off + (s + 1) * 128]

            # DMA issue order for the stream-in phase (few fat DMAs);
            # pos index doubles as the availability ordinal below
            order = ["w1:0", "x:0", "x:1", "g:0", "w1:1", "x:2", "g:1",
                     "x:3", "g:2", "w1:2", "x:4", "x:5"]
            pos = {}
            for p, item in enumerate(order):
                kind, idx = item.split(":")
                {"x": dma_x, "w1": dma_w1, "g": dma_gp}[kind](int(idx))
                pos[item] = p
            # prefetch all remaining gpack pairs (all-resident, no ring waits)
            for p in range(3, NG // 2):
                dma_gp(p)

            def xpos(i):
                for j, (lo, hi) in enumerate(XCH):
                    if lo <= i < hi:
                        return pos[f"x:{j}"]

            def w1pos(slot):
                for k, (lo, hi) in enumerate(W1CH):
                    if lo <= slot < hi:
                        return pos[f"w1:{k}"]

            u_ps = [upsum.tile([128, TPC], dt.float32, tag=f"u{lh}",
                               name=f"ups{lh}") for lh in range(2)]

            # merged emission: u matmuls + lead-group butterfly matmuls,
            # sorted by the DMA position that unblocks them
            events = []
            for i in range(NXT):
                av = max(xpos(i), w1pos(i * 2 + 1))
                events.append((av, 0, ("u", i)))
            for g in range(LEADS):
                gav = pos[f"g:{g // 2}"]
                slots = sorted(
                    range(10),
                    key=lambda s: (max(xpos(int(xtile_idx[g, s])), gav), s))
                first = True
                for s in slots:
                    av = max(xpos(int(xtile_idx[g, s])), gav)
                    events.append((av, 1, ("bf", g, s, first)))
                    first = False
            events.sort(key=lambda e: (e[0], e[1]))

            for av, pri, ev in events:
                if ev[0] == "u":
                    i = ev[1]
                    for lh in range(2):
                        nc.tensor.matmul(u_ps[lh][:], w1slice(i * 2 + lh),
                                         xslice(i),
                                         start=(i == 0), stop=(i == NXT - 1))
                else:
                    _, g, s, first = ev
                    if accs[g] is None:
                        accs[g] = gpsum.tile([128, TPC], dt.float32,
                                             tag="acc", name=f"acc{g}")
                    nc.tensor.matmul(accs[g][:], gslice(g, s),
                                     xslice(int(xtile_idx[g, s])),
                                     start=first, stop=False)

            u_sb = []
            for lh in range(2):
                ut = res_pool.tile([128, TPC], dt.float16, tag=f"usb{lh}",
                                   name=f"usb{lh}")
                nc.vector.tensor_copy(ut[:], u_ps[lh][:])
                u_sb.append(ut)

            ych_of = {}
            for ci, (lo, hi) in enumerate(YCH):
                for g in range(lo, hi):
                    ych_of[g] = ci
            ycur = [None]

            def close_group(g):
                for lh in range(2):
                    nc.tensor.matmul(accs[g][:], gslice(g, 10 + lh),
                                     u_sb[lh][:],
                                     start=False, stop=(lh == 1))
                ci = ych_of[g]
                lo, hi = YCH[ci]
                if g == lo:
                    ycur[0] = res_pool.tile([128, (hi - lo) * TPC],
                                            dt.float16, tag=f"y{ci}",
                                            name=f"yc{ci}")
                c = g - lo
                nc.vector.tensor_scalar_add(
                    ycur[0][:, c * TPC:(c + 1) * TPC], accs[g][:],
                    bt[:, g:g + 1])
                if g == hi - 1:
                    nc.sync.dma_start(y_d[:, lo * TPC:hi * TPC], ycur[0][:])

            for g in range(LEADS):
                close_group(g)

            for g in range(LEADS, NG):
                accs[g] = gpsum.tile([128, TPC], dt.float32, tag="acc",
                                     name=f"acc{g}")
                for s in range(10):
                    nc.tensor.matmul(accs[g][:], gslice(g, s),
                                     xslice(int(xtile_idx[g, s])),
                                     start=(s == 0), stop=False)
                close_group(g)

    nc.compile()
    return nc


def _pack_weights(weight, w1, w2, b, flat):
    r2 = np.arange(BLOCK)
    gpack = np.empty((NG, 128, NSLOT * 128), np.float16)
    # packed below into pairs [NG//2, 128, 2*NSLOT*128] for 6KB DMA rows
    for ob in range(NB):
        for j in range(ACTIVE):
            m = int(flat[ob, j])
            q, a2 = m // ACTIVE, m % ACTIVE
            k = a2 * BLOCK + r2
            Wblk = weight[q * BLOCK + k // ACTIVE, k % ACTIVE, :]  # [r2, c]
            for rh in range(2):
                g = ob * 2 + rh
                for kh in range(2):
                    s = j * 2 + kh
                    gpack[g, :, s * 128:(s + 1) * 128] = \
                        Wblk[rh * 128:(rh + 1) * 128,
                             kh * 128:(kh + 1) * 128].T
    for g in range(NG):
        for lh in range(2):
            s = 10 + lh
            gpack[g, :, s * 128:(s + 1) * 128] = \
                w2[g * 128:(g + 1) * 128, lh * 128:(lh + 1) * 128].T
    gpairs = np.ascontiguousarray(
        gpack.reshape(NG // 2, 2, 128, NSLOT * 128)
             .transpose(0, 2, 1, 3)
             .reshape(NG // 2, 128, 2 * NSLOT * 128))
    w1sb = np.ascontiguousarray(
        w1.reshape(2, 128, 32, 128).transpose(2, 0, 3, 1)
          .reshape(64, 128, 128).transpose(1, 0, 2)
          .reshape(128, 64 * 128)).astype(np.float16)
    bpack = np.ascontiguousarray(b.reshape(NG, 128).T)
    return gpairs, w1sb, bpack


def _ensure_axon_hooks():
    # Some images lack antenv.axon_hooks; bass_utils imports it on the
    # trace path. Provide a stub so trace degrades gracefully.
    import sys
    import types
    try:
        import antenv.axon_hooks  # noqa: F401
        return
    except ImportError:
        pass
    mod = types.ModuleType("antenv.axon_hooks")
    mod._hook = None
    mod.set_axon_ntff_profile_hook = lambda h: setattr(mod, "_hook", h)
    mod.get_axon_ntff_profile_hook = lambda: mod._hook
    sys.modules["antenv.axon_hooks"] = mod
    try:
        import antenv
        antenv.axon_hooks = mod
    except ImportError:
        pass


def kernel(x, weight, w1, w2, b, butterfly_flat_indices):
    _ensure_axon_hooks()
    from concourse.bass_utils import run_bass_kernel_spmd

    x = np.ascontiguousarray(x, np.float32)
    weight = np.ascontiguousarray(weight, np.float32)
    w1 = np.ascontiguousarray(w1, np.float32)
    w2 = np.ascontiguousarray(w2, np.float32)
    b = np.ascontiguousarray(b, np.float32)
    flat = np.asarray(butterfly_flat_indices)

    xtile_idx = _derive_xtile_idx(flat)
    key = xtile_idx.tobytes()
    if key not in _CACHE:
        _CACHE[key] = _build(xtile_idx)
    nc = _CACHE[key]

    gpairs, w1sb, bpack = _pack_weights(weight, w1, w2, b, flat)
    in_maps = []
    for c in range(NCORES):
        xs = x[c * TPC:(c + 1) * TPC]
        xpack = np.ascontiguousarray(
            xs.T.reshape(NXT, 128, TPC).transpose(1, 0, 2)
              .reshape(128, NXT * TPC)).astype(np.float16)
        in_maps.append({"xpack": xpack, "w1pack": w1sb, "gpack": gpairs,
                        "bpack": bpack})

    trace = bool(int(os.environ.get("PIXELFLY_TRACE", "0")))
    res = run_bass_kernel_spmd(nc, in_maps, list(range(NCORES)), trace=trace)
    LAST["exec_time_ns"] = res.exec_time_ns
    LAST["results"] = res

    out = np.empty((TOKENS, OUT_F), np.float32)
    for c in range(NCORES):
        yc = res.results[c]["y"]  # [128, NG*TPC] fp16
        yfull = (yc.reshape(128, NG, TPC).transpose(1, 0, 2)
                   .reshape(OUT_F, TPC))
        out[c * TPC:(c + 1) * TPC] = yfull.T.astype(np.float32)
    return out
